# revision 19
# baseline (speedup 1.0000x reference)
"""Trainium2 Bass kernel for nn_ASIC_44186623541335.

Soft-logic-gate cellular automaton: 8 layers over a 16384-cell ring; each cell
update is a 32-combo soft-boolean match over its 5-neighborhood, weighted by
sigmoid gates and clipped to [0,1].

Algorithm: the per-cell update  sum_c tw[c,n] * prod_i mu(bit_ci, v_i)  is the
multilinear extension of the gate vector tw[:,n] at the 5 neighbor values.
Per layer the gate vector is first Moebius-transformed (a fixed 32x32 +/-1
matrix, done on the TensorEngine) into monomial coefficients A_S[n]; the
evaluation is then a 31-node Horner tree -- each node one multiply + one add,
no subtracts -- batched per level into 2 vector instructions.

Sharding: grid axis split across 8 cores (2048 cells each + 16-cell shrinking
halo, no inter-core communication). Per core the slice lives as 128 partitions
x 17 cells, free dim = (cell, batch) = 21*32 with a 2-cell halo per partition
refreshed between layers by two SBUF-SBUF DMAs; the outermost halo decays into
the redundant 16-cell margin.
"""

import numpy as np

import concourse.bacc as bacc
import concourse.mybir as mybir
from concourse import bass_utils
from concourse.tile import TileContext

GRID = 16384
LAYERS = 8
NPOS = 32
BATCH = 32
M = 4096
STRIDE = 4
NCORES = 8
CORE_N = GRID // NCORES  # 2048

P = 128          # partitions (grid chunks per core)
W = 17           # output cells per partition
WP = W + 4       # stored cells per partition (2-cell halo each side)
HALO = 2 * LAYERS  # 16: core-level shrinking margin
SPAN_COLS = P * W  # 2176 cells covered per core

F32 = mybir.dt.float32
ALU = mybir.AluOpType
ACT_SIGMOID = mybir.ActivationFunctionType.Sigmoid

# batch columns handled by GPSIMD (rest on VectorE); 0 disables the split
GP_B = 0

_CACHE = {}


def _moebius_lhsT():
    """lhsT[c, s] = T[s, c]; A_s = sum_c T[s,c] * tw_c  (multilinear coeffs).

    T[s, c] = (-1)^popcount(s & ~c) if (c & ~s) == 0 else 0.
    """
    t = np.zeros((NPOS, NPOS), dtype=np.float32)
    for s in range(NPOS):
        for c in range(NPOS):
            if c & ~s & 31:
                continue
            t[s, c] = (-1.0) ** bin(s & ~c & 31).count("1")
    return np.ascontiguousarray(t.T)


def _build_program(nrep=None):
    """nrep: timing-only variant — wraps the 8-layer body in a hardware
    loop executed nrep times (results are then meaningless; used to
    measure per-layer cost from wall-clock deltas)."""
    import contextlib

    nc = bacc.Bacc("TRN2", target_bir_lowering=False, debug=False)
    s0_d = nc.dram_tensor("s0", [P, WP * BATCH], F32, kind="ExternalInput")
    twc_d = nc.dram_tensor(
        "twc", [LAYERS, NPOS, SPAN_COLS], F32, kind="ExternalInput"
    )
    tm_d = nc.dram_tensor("tmat", [NPOS, NPOS], F32, kind="ExternalInput")
    out_d = nc.dram_tensor("out", [P, W * BATCH], F32, kind="ExternalOutput")

    FB = W * BATCH  # 544: free size of one (cell, batch) plane

    with TileContext(nc) as tc:
        with (
            tc.tile_pool(name="state", bufs=1) as sp,
            tc.tile_pool(name="coef", bufs=2) as cp,
            tc.tile_pool(name="psum", bufs=1, space="PSUM") as pp,
            tc.tile_pool(name="dram", bufs=2, space="DRAM") as dp,
        ):
            s_a = sp.tile([P, WP * BATCH], F32, tag="s_a")
            s_b = sp.tile([P, WP * BATCH], F32, tag="s_b")
            h1 = sp.tile([P, 16 * FB], F32, tag="h1")
            h2 = sp.tile([P, 8 * FB], F32, tag="h2")
            h3 = sp.tile([P, 4 * FB], F32, tag="h3")
            h4 = sp.tile([P, 2 * FB], F32, tag="h4")
            h5 = sp.tile([P, FB], F32, tag="h5")
            tmat = sp.tile([NPOS, NPOS], F32, tag="tmat")
            asb = sp.tile([NPOS, SPAN_COLS], F32, tag="asb")

            # edge-partition halo columns are never DMA-refreshed; zero-fill
            # once so reads stay finite (values only feed the discarded margin)
            nc.gpsimd.memset(s_a[:], 0.0)
            nc.gpsimd.memset(s_b[:], 0.0)

            nc.sync.dma_start(out=s_a[:], in_=s0_d.ap())
            nc.sync.dma_start(out=tmat[:], in_=tm_d.ap())

            def sview(s, d):
                # state view shifted by d cells: (P, BATCH, W), b-major layout
                return s[:].rearrange("p (b w) -> p b w", w=WP)[
                    :, :, 2 + d : 2 + d + W
                ]

            loop_cm = tc.For_i(0, nrep, 1) if nrep else contextlib.nullcontext()
            with loop_cm:
              for layer in range(LAYERS):
                s_in = s_a if layer % 2 == 0 else s_b
                s_out = s_b if layer % 2 == 0 else s_a

                twr = cp.tile([NPOS, SPAN_COLS], F32, tag="twr")
                tws = cp.tile([NPOS, SPAN_COLS], F32, tag="tws")
                ac = cp.tile([P, NPOS * W], F32, tag="ac")
                ps = pp.tile([NPOS, SPAN_COLS], F32, tag="ps")

                nc.sync.dma_start(out=twr[:], in_=twc_d.ap()[layer])
                nc.scalar.activation(tws[:], twr[:], ACT_SIGMOID)

                # Moebius transform on the PE: A = T @ sigmoid(gates)
                for t0 in range(0, SPAN_COLS, 512):
                    t1 = min(t0 + 512, SPAN_COLS)
                    nc.tensor.matmul(
                        ps[:, t0:t1], tmat[:], tws[:, t0:t1], start=True, stop=True
                    )

                # PSUM -> SBUF (ScalarE sits next to PSUM; ACT is idle here),
                # then re-layout to the chunked (p, sigma, u) tiling via a
                # DRAM bounce (a cross-partition gather is not SBUF->SBUF
                # expressible: both APs would need their partition dim first)
                nc.scalar.copy(asb[:], ps[:])
                adram = dp.tile([NPOS, SPAN_COLS], F32, tag="adram")
                nc.sync.dma_start(out=adram[:], in_=asb[:])
                nc.sync.dma_start(
                    out=ac[:].rearrange("p (s u) -> p s u", u=W),
                    in_=adram[:].rearrange("s (p u) -> p s u", u=W),
                )

                # Horner tree: level k pairs adjacent entries with neighbor
                # value v (shift +2,+1,0,-1,-2):  g[j] = even[j] + v * odd[j]
                # Optionally split along batch columns: VectorE gets b<[0,b_s),
                # GPSIMD independently runs the same tree on b in [b_s,32).
                ac3 = ac[:].rearrange("p (j two u) -> p two j u", two=2, u=W)
                levels = [(h1, h2, 8, 1), (h2, h3, 4, 0), (h3, h4, 2, -1)]

                def svw(s, d, b0, b1):
                    return sview(s, d)[:, b0:b1]

                def hview(hs, cnt, b0, b1):
                    r = hs[:].rearrange(
                        "p (j two b u) -> p two j b u", two=2, b=BATCH, u=W
                    )
                    return r[:, 0][:, :, b0:b1], r[:, 1][:, :, b0:b1]

                def tree(eng, b0, b1):
                    bb = b1 - b0
                    v4 = svw(s_in, 2, b0, b1)
                    h1_4 = h1[:].rearrange("p (j b u) -> p j b u", b=BATCH, u=W)[
                        :, :, b0:b1
                    ]
                    eng.tensor_tensor(
                        h1_4,
                        ac3[:, 1][:, :, None].broadcast_to([P, 16, bb, W]),
                        v4[:, None].broadcast_to([P, 16, bb, W]),
                        ALU.mult,
                    )
                    eng.tensor_tensor(
                        h1_4,
                        h1_4,
                        ac3[:, 0][:, :, None].broadcast_to([P, 16, bb, W]),
                        ALU.add,
                    )
                    for hs, hd, cnt, d in levels:
                        he, ho = hview(hs, cnt, b0, b1)
                        hdv = hd[:].rearrange("p (j b u) -> p j b u", b=BATCH, u=W)[
                            :, :, b0:b1
                        ]
                        vv = svw(s_in, d, b0, b1)[:, None].broadcast_to([P, cnt, bb, W])
                        eng.tensor_tensor(hdv, ho, vv, ALU.mult)
                        eng.tensor_tensor(hdv, hdv, he, ALU.add)
                    # last level: h5 = h4_odd * v0; the final add lands straight
                    # in s_out (no clip needed: the result is a convex
                    # combination of sigmoids, always inside (0,1))
                    he, ho = hview(h4, 1, b0, b1)
                    h5v = h5[:].rearrange("p (b u) -> p b u", u=W)[:, b0:b1]
                    vv = svw(s_in, -2, b0, b1)
                    eng.tensor_tensor(h5v, ho[:, 0], vv, ALU.mult)
                    return he[:, 0], h5v

                def l5_add(eng, he, h5v, out_v, u0, u1):
                    eng.tensor_tensor(
                        out_v[:, :, u0:u1], h5v[:, :, u0:u1], he[:, :, u0:u1], ALU.add
                    )

                B_V = BATCH - GP_B
                he_v, h5_v = tree(nc.vector, 0, B_V)
                out_vv = svw(s_out, 0, 0, B_V)
                if GP_B:
                    he_g, h5_g = tree(nc.gpsimd, B_V, BATCH)
                    out_vg = svw(s_out, 0, B_V, BATCH)
                # halo source columns (u 0,1 and 15,16) first, so the halo
                # DMAs overlap the bulk of the final add
                for eng, he_, h5_, ov in (
                    [(nc.vector, he_v, h5_v, out_vv)]
                    + ([(nc.gpsimd, he_g, h5_g, out_vg)] if GP_B else [])
                ):
                    l5_add(eng, he_, h5_, ov, 0, 2)
                    l5_add(eng, he_, h5_, ov, W - 2, W)

                # halo refresh from neighboring partitions (b-major rows)
                r_out = s_out[:].rearrange("p (b w) -> p b w", w=WP)
                nc.sync.dma_start(
                    out=r_out[1:P, :, 0:2], in_=r_out[0 : P - 1, :, W : W + 2]
                )
                nc.sync.dma_start(
                    out=r_out[0 : P - 1, :, W + 2 : W + 4], in_=r_out[1:P, :, 2:4]
                )

                l5_add(nc.vector, he_v, h5_v, out_vv, 2, W - 2)
                if GP_B:
                    l5_add(nc.gpsimd, he_g, h5_g, out_vg, 2, W - 2)

            s_fin = s_a if LAYERS % 2 == 0 else s_b
            nc.sync.dma_start(
                out=out_d.ap().rearrange("p (b u) -> p b u", u=W), in_=sview(s_fin, 0)
            )

    nc.compile()
    return nc


def _shard_inputs(x, toggle_gates):
    """Host-side layout: chunked initial state + per-core gate windows."""
    s0f = np.zeros((BATCH, GRID), dtype=np.float32)
    s0f[:, ::STRIDE] = x
    tmat = _moebius_lhsT()
    in_maps = []
    p_idx = W * np.arange(P)[:, None] + np.arange(WP)[None, :] - 2  # (P, WP)
    for c in range(NCORES):
        a0 = CORE_N * c - HALO
        idx = (a0 + p_idx) % GRID
        s0_core = np.ascontiguousarray(
            s0f[:, idx].transpose(1, 0, 2).reshape(P, WP * BATCH)
        )
        gidx = (a0 + np.arange(SPAN_COLS)) % GRID
        twc_core = np.ascontiguousarray(toggle_gates[:, :, gidx])
        in_maps.append({"s0": s0_core, "twc": twc_core, "tmat": tmat})
    return in_maps


def _unshard(results):
    y = np.empty((BATCH, M), dtype=np.float32)
    ks = np.arange(CORE_N // STRIDE)  # 512 outputs per core
    off = HALO + STRIDE * ks  # position within the core's 2176-cell span
    for c in range(NCORES):
        dump = results[c]["out"].reshape(P, BATCH, W)
        y[:, (CORE_N // STRIDE) * c + ks] = dump[off // W, :, off % W].T
    return y


def kernel(x: np.ndarray, toggle_gates: np.ndarray) -> np.ndarray:
    if "nc" not in _CACHE:
        _CACHE["nc"] = _build_program()
    nc = _CACHE["nc"]
    in_maps = _shard_inputs(
        np.asarray(x, dtype=np.float32), np.asarray(toggle_gates, dtype=np.float32)
    )
    res = bass_utils.run_bass_kernel_spmd(nc, in_maps, core_ids=list(range(NCORES)))
    return _unshard(res.results)
